# revision 2
# baseline (speedup 1.0000x reference)
"""DH-SNN network kernel for Trainium2 (8 NeuronCores, batch-parallel).

v1: wire-optimized. The per-execute cost over the axon tunnel is dominated
by input bytes shipped per call, so:
  - all weights/decay constants are baked into the NEFF via inline_tensor
    (host-precomputed: sigmoid taus, (W1*mask) branch-major reorder,
    2*(1-alpha)*(1-beta_j) prescale, readout time-weights) — shipped once
    at model load, not per execute;
  - only x crosses the wire per call, as fp16 in the matmul-ready layout
    [NSB, 701, BC*TB] (700 input rows + ones-row for bias, T padded to 256).

Device math (per core, batch shard Bc=16), unchanged from baseline:
  DI'[f,(b,t)] = xT @ Wm''  (PE fp16 matmuls, features on partitions)
  d-scan over time per (f,b): d = beta_j*d + DI'  (DVE tensor_tensor_scan,
      reset pattern kills state at per-b series boundaries; cross-superblock
      carry injected into the tau=0 column)
  som''[h,(b,t)] = sum_j d  (Pool adds; som'' = 2*(1-alpha)*som)
  v-loop (w = 2*v): w_t = alpha*w_{t-1} - Y_t;  Y_{t+1} = (w_t >= 1) - som''_{t+1}
  readout: out[b,o] = sum_t c_{t,o} * (s_t @ W2.T)[b,o] + b2[o]*sum_t c_{t,o}
      with c_{t,o} = (1 - gam_o^(250-t))/250; linear in s so P = W2T@(Y+som'')
      accumulates on PE, then weighted time-reduce on DVE.
"""

import os
import hashlib
import numpy as np

B_FULL, T, INP, H, NB, OUT = 128, 250, 700, 512, 4, 20
NCORES = 8
BC = B_FULL // NCORES          # 16 batch per core
TB = 32                        # superblock length
NSB = 8                        # superblocks (Tpad = 256)
TPAD = NSB * TB
F = H * NB                     # 2048 features, branch-major
NM = F // 128                  # 16 feature tiles
KC = 6                         # K chunks (700 inputs + 1 ones-row = 701 rows)
KSZ = [128] * 5 + [61]
NK = INP + 1                   # 701
REPS = int(os.environ.get("SNN_REPS", "1"))
ABL = set(os.environ.get("SNN_ABLATE", "").split(","))

_CACHE = {}


def _sigmoid(z):
    return 1.0 / (1.0 + np.exp(-z))


def _prep_consts(W1, b1, tau_n, tau_m_h, W2, b2, tau_m_ro, mask):
    """Host-side: everything that does not depend on x, in float64."""
    f8 = np.float64
    beta = _sigmoid(np.asarray(tau_n, f8).reshape(NB))
    alpha = float(_sigmoid(float(np.asarray(tau_m_h).reshape(()))))
    gam = _sigmoid(np.asarray(tau_m_ro, f8).reshape(OUT))
    sc = 2.0 * (1.0 - alpha) * (1.0 - beta)            # (NB,)

    # feature reorder g=h*NB+j -> f'=j*H+h
    W1r = np.asarray(W1, f8).reshape(H, NB, INP).transpose(1, 0, 2).reshape(F, INP)
    mkr = np.asarray(mask, f8).reshape(H, NB, INP).transpose(1, 0, 2).reshape(F, INP)
    b1r = np.asarray(b1, f8).reshape(H, NB).T.reshape(F)
    scf = np.repeat(sc, H)                             # (F,)
    Wfull = np.zeros((NK, F), f8)
    Wfull[:INP] = (W1r * mkr).T * scf[None, :]
    Wfull[INP] = b1r * scf                             # bias row (ones-row in x)
    wt16 = Wfull.astype(np.float16)                    # (701, F)

    alpha128 = np.full((128, 1), alpha, np.float32)
    bscale = [np.full((128, 1), beta[j], np.float32) for j in range(NB)]
    # scan reset patterns, one per j-pair: beta_j on its j slice, 0 at tau=0
    pat = []
    for jp in range(2):
        p = np.zeros((128, 2, BC, TB), np.float32)
        for jj in range(2):
            p[:, jj, :, 1:] = beta[2 * jp + jj]
        pat.append(p)

    # readout time weights: column tau of (Y+som'') holds s_{tau-1}; weight
    # by c_{tau-1}: C[tau] = (1 - gam^(T+1-tau))/T for tau in [1, T], else 0
    tau_i = np.arange(TPAD)
    ctau = (1.0 - gam[:, None] ** (T + 1 - tau_i[None, :])) / T
    ctau[:, 0] = 0.0
    ctau[:, T + 1:] = 0.0
    cbig = np.repeat(ctau[:, None, :], BC, axis=1).astype(np.float32)  # (OUT, BC, TPAD)
    b2term = (np.asarray(b2, f8).reshape(OUT) * ctau.sum(1)).reshape(OUT, 1).astype(np.float32)

    w2t = np.ascontiguousarray(np.asarray(W2, np.float32).T).reshape(4, 128, OUT)
    return dict(wt16=wt16, alpha128=alpha128, bscale=bscale, pat=pat,
                cbig=cbig, b2term=b2term, w2t=w2t)


def _build_program(C):
    import concourse.tile as tile
    from concourse import bacc, mybir
    from contextlib import ExitStack

    DT = mybir.dt
    ALU = mybir.AluOpType
    AF = mybir.ActivationFunctionType
    f32 = DT.float32
    f16 = DT.float16

    nc = bacc.Bacc("TRN2", target_bir_lowering=False, debug=False)

    xh_d = nc.dram_tensor("xh", [NSB, NK, BC * TB], f16, kind="ExternalInput")
    out_d = nc.dram_tensor("out", [BC, OUT], f32, kind="ExternalOutput")

    wt_d = nc.inline_tensor(C["wt16"], name="wtc")
    a128_d = nc.inline_tensor(C["alpha128"], name="a128c")
    bsc_d = [nc.inline_tensor(C["bscale"][j], name=f"bscc{j}") for j in range(NB)]
    pat_d = [nc.inline_tensor(C["pat"][jp], name=f"patc{jp}") for jp in range(2)]
    cbig_d = nc.inline_tensor(C["cbig"], name="cbigc")
    b2t_d = nc.inline_tensor(C["b2term"], name="b2tc")
    w2t_d = nc.inline_tensor(C["w2t"], name="w2tc")

    with tile.TileContext(nc) as tc, ExitStack() as ctx:
        const = ctx.enter_context(tc.tile_pool(name="const", bufs=1))
        xt_pool = ctx.enter_context(tc.tile_pool(name="xt", bufs=2))
        d_pool = ctx.enter_context(tc.tile_pool(name="dp", bufs=8))
        tmp_pool = ctx.enter_context(tc.tile_pool(name="tp", bufs=3))
        ps_di = ctx.enter_context(tc.tile_pool(name="psdi", bufs=3, space="PSUM"))
        ps_ro = ctx.enter_context(tc.tile_pool(name="psro", bufs=2, space="PSUM"))

        # ------------- constants to SBUF -------------
        wt = []
        for k in range(KC):
            w_ = const.tile([128, F], f16, tag=f"wt{k}", name=f"wt{k}")
            nc.sync.dma_start(w_[0:KSZ[k], :], wt_d.ap()[k * 128:k * 128 + KSZ[k], :])
            wt.append(w_)
        w2t = []
        for c in range(4):
            w2c = const.tile([128, OUT], f32, tag=f"w2t{c}", name=f"w2t{c}")
            nc.sync.dma_start(w2c[:], w2t_d.ap()[c])
            w2t.append(w2c)
        alpha128 = const.tile([128, 1], f32, tag="a128", name="a128")
        nc.sync.dma_start(alpha128[:], a128_d.ap())
        beta128 = []
        for j in range(NB):
            b_ = const.tile([128, 1], f32, tag=f"b128_{j}", name=f"b128_{j}")
            nc.sync.dma_start(b_[:], bsc_d[j].ap())
            beta128.append(b_)
        pat = []
        for jp in range(2):
            p_ = const.tile([128, 2, BC, TB], f32, tag=f"pat{jp}", name=f"pat{jp}")
            nc.sync.dma_start(p_[:], pat_d[jp].ap())
            pat.append(p_)
        cbig = const.tile([OUT, BC, TPAD], f32, tag="cbig", name="cbig")
        nc.sync.dma_start(cbig[:], cbig_d.ap())
        b2term = const.tile([OUT, 1], f32, tag="b2term", name="b2term")
        nc.sync.dma_start(b2term[:], b2t_d.ap())

        # ------------- state -------------
        carry = const.tile([128, NM, BC], f32, tag="carry", name="carry")
        nc.gpsimd.memset(carry[:], 0.0)
        wv = const.tile([128, 4, BC], f32, tag="wv", name="wv")
        nc.gpsimd.memset(wv[:], 0.0)
        acc = const.tile([OUT, BC], f32, tag="acc", name="acc")
        nc.gpsimd.memset(acc[:], 0.0)
        som = [const.tile([128, 4, BC, TB], f32, tag=f"som{i}", name=f"som{i}") for i in range(2)]
        xn = [const.tile([128, 4, BC, TB], f32, tag=f"xn{i}", name=f"xn{i}") for i in range(2)]

        def compute_sb(s):
            """matmuls + scans + branch sum for superblock s."""
            xts = []
            for k in range(KC):
                xt = xt_pool.tile([128, BC * TB], f16, tag=f"xt{k}", name=f"xt{k}")
                nc.sync.dma_start(xt[0:KSZ[k], :], xh_d.ap()[s, k * 128:k * 128 + KSZ[k], :])
                xts.append(xt)
            for c in range(4):
                dt_ = []
                for jp in range(2):
                    pd = ps_di.tile([128, 2, BC * TB], f32, tag="di", name="di")
                    for jj in (() if "mm" in ABL else range(2)):
                        m = (2 * jp + jj) * 4 + c
                        for k in range(KC):
                            nc.tensor.matmul(
                                pd[:, jj],
                                lhsT=wt[k][0:KSZ[k], m * 128:(m + 1) * 128],
                                rhs=xts[k][0:KSZ[k], :],
                                start=(k == 0), stop=(k == KC - 1),
                            )
                    pd4 = pd[:].rearrange("p j (b t) -> p j b t", b=BC)
                    # inject beta-prescaled cross-superblock carries (tau=0)
                    m0 = 2 * jp * 4 + c
                    nc.vector.tensor_tensor(
                        pd4[:, :, :, 0], pd4[:, :, :, 0],
                        carry[:, m0:m0 + 5:4, :], ALU.add)
                    d_ = d_pool.tile([128, 2, BC, TB], f32, tag="d", name="d")
                    nc.vector.tensor_tensor_scan(
                        d_[:].rearrange("p j b t -> p (j b t)"),
                        pat[jp][:].rearrange("p j b t -> p (j b t)"),
                        pd[:].rearrange("p j bt -> p (j bt)"),
                        initial=0.0, op0=ALU.mult, op1=ALU.add)
                    for jj in range(2):
                        m = (2 * jp + jj) * 4 + c
                        nc.scalar.activation(carry[:, m, :], d_[:, jj, :, TB - 1],
                                             AF.Copy, scale=beta128[2 * jp + jj][:])
                    dt_.append(d_)
                t01 = tmp_pool.tile([128, BC, TB], f32, tag="t01", name="t01")
                t23 = tmp_pool.tile([128, BC, TB], f32, tag="t23", name="t23")
                nc.gpsimd.tensor_tensor(t01[:], dt_[0][:, 0], dt_[0][:, 1], ALU.add)
                nc.gpsimd.tensor_tensor(t23[:], dt_[1][:, 0], dt_[1][:, 1], ALU.add)
                nc.gpsimd.tensor_tensor(som[s % 2][:, c], t01[:], t23[:], ALU.add)

        def vloop(s):
            if "vloop" in ABL:
                return
            if s == 0:
                nc.vector.tensor_scalar(xn[0][:, :, :, 0], som[0][:, :, :, 0],
                                        -1.0, None, ALU.mult)
            for tl in range(TB):
                t = s * TB + tl
                if t > T - 1:
                    break
                nc.vector.scalar_tensor_tensor(
                    wv[:], wv[:], alpha128[:],
                    xn[s % 2][:, :, :, tl],
                    op0=ALU.mult, op1=ALU.subtract)
                tn = t + 1
                if tn <= T:
                    s2, tl2 = divmod(tn, TB)
                    nc.vector.scalar_tensor_tensor(
                        xn[s2 % 2][:, :, :, tl2], wv[:], 1.0,
                        som[s2 % 2][:, :, :, tl2],
                        op0=ALU.is_ge, op1=ALU.subtract)

        def readout(s):
            if "ro" in ABL:
                return
            P = ps_ro.tile([OUT, BC * TB], f32, tag="P", name="P")
            first = True
            for c in range(4):
                for src in (xn[s % 2], som[s % 2]):
                    s2d = src[:].rearrange("p c b t -> p (c b t)")
                    nc.tensor.matmul(
                        P[:],
                        lhsT=w2t[c][:],
                        rhs=s2d[:, c * BC * TB:(c + 1) * BC * TB],
                        start=first, stop=(c == 3 and src is som[s % 2]))
                    first = False
            p3 = P[:].rearrange("p (b t) -> p b t", b=BC)
            nc.vector.tensor_tensor(
                p3, p3, cbig[:, :, s * TB:(s + 1) * TB], ALU.mult)
            res = tmp_pool.tile([OUT, BC], f32, tag="res", name="res")
            nc.vector.tensor_reduce(res[:], p3, axis=mybir.AxisListType.X,
                                    op=ALU.add)
            nc.vector.tensor_tensor(acc[:], acc[:], res[:], ALU.add)

        for _rep in range(REPS):
            compute_sb(0)
            compute_sb(1)
            for s in range(NSB):
                vloop(s)
                readout(s)
                if s + 2 < NSB:
                    compute_sb(s + 2)

        final = const.tile([OUT, BC], f32, tag="final", name="final")
        nc.vector.tensor_scalar(final[:], acc[:], b2term[:], None, ALU.add)
        nc.sync.dma_start(out_d.ap().rearrange("b o -> o b"), final[:])

    nc.compile()
    return nc


class _Exec:
    """Holds the compiled program and a reusable jitted SPMD dispatcher."""

    def __init__(self, nc):
        import jax
        from jax.sharding import Mesh, PartitionSpec
        from jax.experimental.shard_map import shard_map
        from concourse import bass2jax, mybir

        self.nc = nc
        bass2jax.install_neuronx_cc_hook()
        in_names, out_names, out_avals = [], [], []
        for alloc in nc.m.functions[0].allocations:
            if not isinstance(alloc, mybir.MemoryLocationSet):
                continue
            name = alloc.memorylocations[0].name
            pname = nc.partition_id_tensor.name if nc.partition_id_tensor else None
            if alloc.kind == "ExternalInput":
                if name != pname:
                    in_names.append(name)
            elif alloc.kind == "ExternalOutput":
                out_names.append(name)
                shape = tuple(alloc.tensor_shape)
                dtype = mybir.dt.np(alloc.dtype)
                out_avals.append(jax.core.ShapedArray(shape, dtype))
        self.in_names = list(in_names)
        self.out_names = list(out_names)
        self.out_avals = list(out_avals)
        all_names = in_names + out_names
        if nc.partition_id_tensor is not None:
            all_names = all_names + [nc.partition_id_tensor.name]
        n_io = len(in_names) + len(out_names)
        out_avals_t = tuple(out_avals)
        out_names_t = tuple(out_names)
        all_names_t = tuple(all_names)

        def _body(*args):
            operands = list(args)
            if nc.partition_id_tensor is not None:
                operands.append(bass2jax.partition_id_tensor())
            outs = bass2jax._bass_exec_p.bind(
                *operands, out_avals=out_avals_t, in_names=all_names_t,
                out_names=out_names_t, lowering_input_output_aliases=(),
                sim_require_finite=True, sim_require_nnan=True, nc=nc)
            return tuple(outs)

        devices = jax.devices()[:NCORES]
        self.mesh = Mesh(np.asarray(devices), ("core",))
        self.sharded = jax.jit(
            shard_map(_body, mesh=self.mesh,
                      in_specs=(PartitionSpec("core"),) * n_io,
                      out_specs=(PartitionSpec("core"),) * len(out_names),
                      check_rep=False),
            keep_unused=True)

    def zeros_out(self):
        from concourse import mybir
        return [np.zeros((NCORES * a.shape[0], *a.shape[1:]), a.dtype)
                for a in self.out_avals]

    def run(self, xh_concat):
        import jax
        out = self.sharded(xh_concat, *self.zeros_out())
        jax.block_until_ready(out)
        return np.asarray(out[0])


def _weights_sig(ws):
    h = hashlib.md5()
    for a in ws:
        a = np.ascontiguousarray(np.asarray(a))
        h.update(str(a.shape).encode())
        h.update(str(a.dtype).encode())
        h.update(a.tobytes())
    return h.hexdigest()


def get_exec(W1, b1, tau_n, tau_m_h, W2, b2, tau_m_ro, mask):
    sig = _weights_sig([W1, b1, tau_n, tau_m_h, W2, b2, tau_m_ro, mask])
    ex = _CACHE.get(sig)
    if ex is None:
        C = _prep_consts(W1, b1, tau_n, tau_m_h, W2, b2, tau_m_ro, mask)
        nc = _build_program(C)
        ex = _Exec(nc)
        _CACHE[sig] = ex
    return ex


def marshal_x(x):
    """(B, T, IN) f32 -> concatenated (NCORES*NSB, NK, BC*TB) fp16 wire tensor."""
    x = np.asarray(x, np.float32)
    xp = np.zeros((B_FULL, TPAD, NK), np.float16)
    xp[:, :T, :INP] = x.astype(np.float16)
    xp[:, :, INP] = 1.0     # ones-row for bias
    # (B, TPAD, NK) -> (NCORES, NSB, NK, BC*TB)
    xpc = xp.reshape(NCORES, BC, NSB, TB, NK)
    xh = np.ascontiguousarray(xpc.transpose(0, 2, 4, 1, 3)).reshape(
        NCORES * NSB, NK, BC * TB)
    return xh


def kernel(x, W1, b1, tau_n, tau_m_h, W2, b2, tau_m_ro, mask):
    ex = get_exec(W1, b1, tau_n, tau_m_h, W2, b2, tau_m_ro, mask)
    xh = marshal_x(x)
    out = ex.run(xh)                        # (NCORES*BC, OUT)
    return out.reshape(B_FULL, OUT)


# revision 15
# speedup vs baseline: 1.1463x; 1.1463x over previous
"""DH-SNN network kernel for Trainium2 (8 NeuronCores, batch-parallel).

Wire-optimized: the per-execute cost over the axon tunnel is dominated by
input bytes shipped per call, so:
  - all weights/decay constants are baked into the NEFF via inline_tensor
    (host-precomputed: sigmoid taus, (W1*mask) branch-major reorder,
    2*(1-alpha)*(1-beta_j) prescale, readout time-weights) — shipped once
    at model load, not per execute;
  - only x crosses the wire per call, as uint8 (round(x*255)) in the
    matmul-ready layout [NSB, 701, BC*TB] (700 input rows + a 255-row for
    the bias, T padded to 256); on device it is widened to fp16 (ACT copy)
    and the 1/255 dequant is applied once at the linear branch-sum exit
    (not folded into the fp16 weights, where it would hit denormal flush).

Device math (per core, batch shard Bc=16), unchanged from baseline:
  DI'[f,(b,t)] = xT @ Wm''  (PE fp16 matmuls, features on partitions)
  d-scan over time per (f,b): d = beta_j*d + DI'  (DVE tensor_tensor_scan,
      reset pattern kills state at per-b series boundaries; cross-superblock
      carry injected into the tau=0 column)
  som''[h,(b,t)] = sum_j d  (Pool adds; som'' = 2*(1-alpha)*som)
  v-loop (w = 2*v): w_t = alpha*w_{t-1} - Y_t;  Y_{t+1} = (w_t >= 1) - som''_{t+1}
  readout: out[b,o] = sum_t c_{t,o} * (s_t @ W2.T)[b,o] + b2[o]*sum_t c_{t,o}
      with c_{t,o} = (1 - gam_o^(250-t))/250; linear in s so P = W2T@(Y+som'')
      accumulates on PE, then weighted time-reduce on DVE.
"""

import os
import hashlib
import numpy as np

B_FULL, T, INP, H, NB, OUT = 128, 250, 700, 512, 4, 20
NCORES = 8
BC = B_FULL // NCORES          # 16 batch per core
TB = 32                        # superblock length
NSB = 8                        # superblocks (Tpad = 256)
TPAD = NSB * TB
F = H * NB                     # 2048 features, branch-major
NM = F // 128                  # 16 feature tiles
KC = 6                         # K chunks (700 inputs + 1 ones-row = 701 rows)
KSZ = [128] * 5 + [61]
NK = INP + 1                   # 701
REPS = int(os.environ.get("SNN_REPS", "1"))
ABL = set(os.environ.get("SNN_ABLATE", "").split(","))

_CACHE = {}


def _sigmoid(z):
    return 1.0 / (1.0 + np.exp(-z))


def _prep_consts(W1, b1, tau_n, tau_m_h, W2, b2, tau_m_ro, mask):
    """Host-side: everything that does not depend on x, in float64."""
    f8 = np.float64
    beta = _sigmoid(np.asarray(tau_n, f8).reshape(NB))
    alpha = float(_sigmoid(float(np.asarray(tau_m_h).reshape(()))))
    gam = _sigmoid(np.asarray(tau_m_ro, f8).reshape(OUT))
    sc = 2.0 * (1.0 - alpha) * (1.0 - beta)            # (NB,)

    # feature reorder g=h*NB+j -> f'=j*H+h
    W1r = np.asarray(W1, f8).reshape(H, NB, INP).transpose(1, 0, 2).reshape(F, INP)
    mkr = np.asarray(mask, f8).reshape(H, NB, INP).transpose(1, 0, 2).reshape(F, INP)
    b1r = np.asarray(b1, f8).reshape(H, NB).T.reshape(F)
    scf = np.repeat(sc, H)                             # (F,)
    # x ships as uint8 (round(x*255)) and is DMA-cast to fp16 holding 0..255.
    # Weights stay unscaled (a /255 fold would push ~1/3 of nonzero weights
    # into fp16 denormals, which the PE flushes); the dendritic scan is linear
    # in DI, so the 1/255 dequant is applied once at the branch-sum exit.
    Wfull = np.zeros((NK, F), f8)
    Wfull[:INP] = (W1r * mkr).T * scf[None, :]
    Wfull[INP] = b1r * scf                             # bias row (255-row in x)
    wt16 = Wfull.astype(np.float16)                    # (701, F)

    alpha128 = np.full((128, 1), alpha, np.float32)
    bscale = [np.full((128, 1), beta[j], np.float32) for j in range(NB)]
    # scan reset patterns, one per j-pair: beta_j on its j slice, 0 at tau=0
    pat = []
    for jp in range(2):
        p = np.zeros((128, 2, BC, TB), np.float32)
        for jj in range(2):
            p[:, jj, :, 1:] = beta[2 * jp + jj]
        pat.append(p)

    # readout time weights: column tau of (Y+som'') holds s_{tau-1}; weight
    # by c_{tau-1}: C[tau] = (1 - gam^(T+1-tau))/T for tau in [1, T], else 0
    tau_i = np.arange(TPAD)
    ctau = (1.0 - gam[:, None] ** (T + 1 - tau_i[None, :])) / T
    ctau[:, 0] = 0.0
    ctau[:, T + 1:] = 0.0
    cbig = np.repeat(ctau[:, None, :], BC, axis=1).astype(np.float32)  # (OUT, BC, TPAD)
    b2term = (np.asarray(b2, f8).reshape(OUT) * ctau.sum(1)).reshape(OUT, 1).astype(np.float32)

    w2t = np.ascontiguousarray(np.asarray(W2, np.float32).T).reshape(4, 128, OUT)
    return dict(wt16=wt16, alpha128=alpha128, bscale=bscale, pat=pat,
                cbig=cbig, b2term=b2term, w2t=w2t)


def _build_program(C, cast="act", ro_r=False, vl_split=False):
    import concourse.tile as tile
    from concourse import bacc, mybir
    from contextlib import ExitStack

    DT = mybir.dt
    ALU = mybir.AluOpType
    AF = mybir.ActivationFunctionType
    f32 = DT.float32
    f16 = DT.float16

    nc = bacc.Bacc("TRN2", target_bir_lowering=False, debug=False)

    xh_d = nc.dram_tensor("xh", [NSB, NK, BC * TB], DT.uint8, kind="ExternalInput")
    out_d = nc.dram_tensor("out", [BC, OUT], f32, kind="ExternalOutput")

    wt_d = nc.inline_tensor(C["wt16"], name="wtc")
    a128_d = nc.inline_tensor(C["alpha128"], name="a128c")
    bsc_d = [nc.inline_tensor(C["bscale"][j], name=f"bscc{j}") for j in range(NB)]
    pat_d = [nc.inline_tensor(C["pat"][jp], name=f"patc{jp}") for jp in range(2)]
    cbig_d = nc.inline_tensor(C["cbig"], name="cbigc")
    b2t_d = nc.inline_tensor(C["b2term"], name="b2tc")
    w2t_d = nc.inline_tensor(C["w2t"], name="w2tc")

    with tile.TileContext(nc) as tc, ExitStack() as ctx:
        const = ctx.enter_context(tc.tile_pool(name="const", bufs=1))
        xt_pool = ctx.enter_context(tc.tile_pool(name="xt", bufs=2))
        d_pool = ctx.enter_context(tc.tile_pool(name="dp", bufs=8))
        tmp_pool = ctx.enter_context(tc.tile_pool(name="tp", bufs=3))
        ps_di = ctx.enter_context(tc.tile_pool(name="psdi", bufs=3, space="PSUM"))
        ps_ro = ctx.enter_context(tc.tile_pool(name="psro", bufs=2, space="PSUM"))

        # ------------- constants to SBUF -------------
        wt = []
        for k in range(KC):
            w_ = const.tile([128, F], f16, tag=f"wt{k}", name=f"wt{k}")
            nc.sync.dma_start(w_[0:KSZ[k], :], wt_d.ap()[k * 128:k * 128 + KSZ[k], :])
            wt.append(w_)
        w2t = []
        for c in range(4):
            w2c = const.tile([128, OUT], f32, tag=f"w2t{c}", name=f"w2t{c}")
            nc.sync.dma_start(w2c[:], w2t_d.ap()[c])
            w2t.append(w2c)
        alpha128 = const.tile([128, 1], f32, tag="a128", name="a128")
        nc.sync.dma_start(alpha128[:], a128_d.ap())
        beta128 = []
        for j in range(NB):
            b_ = const.tile([128, 1], f32, tag=f"b128_{j}", name=f"b128_{j}")
            nc.sync.dma_start(b_[:], bsc_d[j].ap())
            beta128.append(b_)
        pat = []
        for jp in range(2):
            p_ = const.tile([128, 2, BC, TB], f32, tag=f"pat{jp}", name=f"pat{jp}")
            nc.sync.dma_start(p_[:], pat_d[jp].ap())
            pat.append(p_)
        cbig = const.tile([OUT, BC, TPAD], f32, tag="cbig", name="cbig")
        nc.sync.dma_start(cbig[:], cbig_d.ap())
        b2term = const.tile([OUT, 1], f32, tag="b2term", name="b2term")
        nc.sync.dma_start(b2term[:], b2t_d.ap())

        # ------------- state -------------
        carry = const.tile([128, NM, BC], f32, tag="carry", name="carry")
        nc.gpsimd.memset(carry[:], 0.0)
        wv = const.tile([128, 4, BC], f32, tag="wv", name="wv")
        nc.gpsimd.memset(wv[:], 0.0)
        acc = const.tile([OUT, BC], f32, tag="acc", name="acc")
        nc.gpsimd.memset(acc[:], 0.0)
        som = [const.tile([128, 4, BC, TB], f32, tag=f"som{i}", name=f"som{i}") for i in range(2)]
        xn = [const.tile([128, 4, BC, TB], f32, tag=f"xn{i}", name=f"xn{i}") for i in range(2)]

        def compute_sb(s):
            """matmuls + scans + branch sum for superblock s."""
            xts = []
            for k in range(KC):
                xt = xt_pool.tile([128, BC * TB], f16, tag=f"xt{k}", name=f"xt{k}")
                if cast == "swdge":
                    # gpsimd (software DGE) casts uint8 -> fp16 in-flight
                    nc.gpsimd.dma_start(xt[0:KSZ[k], :],
                                        xh_d.ap()[s, k * 128:k * 128 + KSZ[k], :])
                else:
                    xu = xt_pool.tile([128, BC * TB], DT.uint8,
                                      tag=f"xu{k}", name=f"xu{k}")
                    nc.sync.dma_start(xu[0:KSZ[k], :],
                                      xh_d.ap()[s, k * 128:k * 128 + KSZ[k], :])
                    if cast == "act":
                        nc.scalar.activation(xt[0:KSZ[k], :], xu[0:KSZ[k], :], AF.Copy)
                    else:
                        nc.vector.tensor_copy(xt[0:KSZ[k], :], xu[0:KSZ[k], :])
                xts.append(xt)
            for c in range(4):
                dt_ = []
                for jp in range(2):
                    pd = ps_di.tile([128, 2, BC * TB], f32, tag="di", name="di")
                    for jj in (() if "mm" in ABL else range(2)):
                        m = (2 * jp + jj) * 4 + c
                        for k in range(KC):
                            nc.tensor.matmul(
                                pd[:, jj],
                                lhsT=wt[k][0:KSZ[k], m * 128:(m + 1) * 128],
                                rhs=xts[k][0:KSZ[k], :],
                                start=(k == 0), stop=(k == KC - 1),
                            )
                    pd4 = pd[:].rearrange("p j (b t) -> p j b t", b=BC)
                    # inject beta-prescaled cross-superblock carries (tau=0)
                    m0 = 2 * jp * 4 + c
                    nc.vector.tensor_tensor(
                        pd4[:, :, :, 0], pd4[:, :, :, 0],
                        carry[:, m0:m0 + 5:4, :], ALU.add)
                    d_ = d_pool.tile([128, 2, BC, TB], f32, tag="d", name="d")
                    nc.vector.tensor_tensor_scan(
                        d_[:].rearrange("p j b t -> p (j b t)"),
                        pat[jp][:].rearrange("p j b t -> p (j b t)"),
                        pd[:].rearrange("p j bt -> p (j bt)"),
                        initial=0.0, op0=ALU.mult, op1=ALU.add)
                    for jj in range(2):
                        m = (2 * jp + jj) * 4 + c
                        nc.scalar.activation(carry[:, m, :], d_[:, jj, :, TB - 1],
                                             AF.Copy, scale=beta128[2 * jp + jj][:])
                    dt_.append(d_)
                t01 = tmp_pool.tile([128, BC, TB], f32, tag="t01", name="t01")
                t23 = tmp_pool.tile([128, BC, TB], f32, tag="t23", name="t23")
                nc.gpsimd.tensor_tensor(t01[:], dt_[0][:, 0], dt_[0][:, 1], ALU.add)
                nc.gpsimd.tensor_tensor(t23[:], dt_[1][:, 0], dt_[1][:, 1], ALU.add)
                nc.gpsimd.tensor_tensor(t01[:], t01[:], t23[:], ALU.add)
                # dequant: d carries the 255x from the uint8 x wire format
                nc.gpsimd.tensor_scalar(som[s % 2][:, c], t01[:], 1.0 / 255.0,
                                        None, ALU.mult)

        if vl_split:
            veng = [nc.vector, nc.gpsimd]
            vsl = [slice(0, 2), slice(2, 4)]
        else:
            veng = [nc.vector]
            vsl = [slice(0, 4)]

        def vloop(s):
            if "vloop" in ABL:
                return
            if s == 0:
                for eng, cs in zip(veng, vsl):
                    eng.tensor_scalar(xn[0][:, cs, :, 0], som[0][:, cs, :, 0],
                                      -1.0, None, ALU.mult)
            for tl in range(TB):
                t = s * TB + tl
                if t > T - 1:
                    break
                for eng, cs in zip(veng, vsl):
                    eng.scalar_tensor_tensor(
                        wv[:, cs, :], wv[:, cs, :], alpha128[:],
                        xn[s % 2][:, cs, :, tl],
                        op0=ALU.mult, op1=ALU.subtract)
                tn = t + 1
                if tn <= T:
                    s2, tl2 = divmod(tn, TB)
                    for eng, cs in zip(veng, vsl):
                        eng.scalar_tensor_tensor(
                            xn[s2 % 2][:, cs, :, tl2], wv[:, cs, :], 1.0,
                            som[s2 % 2][:, cs, :, tl2],
                            op0=ALU.is_ge, op1=ALU.subtract)

        def readout(s):
            if "ro" in ABL:
                return
            P = ps_ro.tile([OUT, BC * TB], f32, tag="P", name="P")
            first = True
            for c in range(4):
                for src in (xn[s % 2], som[s % 2]):
                    s2d = src[:].rearrange("p c b t -> p (c b t)")
                    nc.tensor.matmul(
                        P[:],
                        lhsT=w2t[c][:],
                        rhs=s2d[:, c * BC * TB:(c + 1) * BC * TB],
                        start=first, stop=(c == 3 and src is som[s % 2]))
                    first = False
            p3 = P[:].rearrange("p (b t) -> p b t", b=BC)
            nc.vector.tensor_tensor(
                p3, p3, cbig[:, :, s * TB:(s + 1) * TB], ALU.mult)
            res = tmp_pool.tile([OUT, BC], f32, tag="res", name="res")
            nc.vector.tensor_reduce(res[:], p3, axis=mybir.AxisListType.X,
                                    op=ALU.add)
            nc.vector.tensor_tensor(acc[:], acc[:], res[:], ALU.add)

        for _rep in range(REPS):
            compute_sb(0)
            compute_sb(1)
            for s in range(NSB):
                vloop(s)
                readout(s)
                if s + 2 < NSB:
                    compute_sb(s + 2)

        final = const.tile([OUT, BC], f32, tag="final", name="final")
        nc.vector.tensor_scalar(final[:], acc[:], b2term[:], None, ALU.add)
        nc.sync.dma_start(out_d.ap().rearrange("b o -> o b"), final[:])

    nc.compile()
    return nc


class _Exec:
    """Holds the compiled program and a reusable jitted SPMD dispatcher."""

    def __init__(self, nc):
        import jax
        from jax.sharding import Mesh, PartitionSpec
        from jax.experimental.shard_map import shard_map
        from concourse import bass2jax, mybir

        self.nc = nc
        bass2jax.install_neuronx_cc_hook()
        in_names, out_names, out_avals = [], [], []
        for alloc in nc.m.functions[0].allocations:
            if not isinstance(alloc, mybir.MemoryLocationSet):
                continue
            name = alloc.memorylocations[0].name
            pname = nc.partition_id_tensor.name if nc.partition_id_tensor else None
            if alloc.kind == "ExternalInput":
                if name != pname:
                    in_names.append(name)
            elif alloc.kind == "ExternalOutput":
                out_names.append(name)
                shape = tuple(alloc.tensor_shape)
                dtype = mybir.dt.np(alloc.dtype)
                out_avals.append(jax.core.ShapedArray(shape, dtype))
        self.in_names = list(in_names)
        self.out_names = list(out_names)
        self.out_avals = list(out_avals)
        all_names = in_names + out_names
        if nc.partition_id_tensor is not None:
            all_names = all_names + [nc.partition_id_tensor.name]
        n_io = len(in_names) + len(out_names)
        out_avals_t = tuple(out_avals)
        out_names_t = tuple(out_names)
        all_names_t = tuple(all_names)

        def _body(*args):
            operands = list(args)
            if nc.partition_id_tensor is not None:
                operands.append(bass2jax.partition_id_tensor())
            outs = bass2jax._bass_exec_p.bind(
                *operands, out_avals=out_avals_t, in_names=all_names_t,
                out_names=out_names_t, lowering_input_output_aliases=(),
                sim_require_finite=True, sim_require_nnan=True, nc=nc)
            return tuple(outs)

        devices = jax.devices()[:NCORES]
        self.mesh = Mesh(np.asarray(devices), ("core",))
        self.sharded = jax.jit(
            shard_map(_body, mesh=self.mesh,
                      in_specs=(PartitionSpec("core"),) * n_io,
                      out_specs=(PartitionSpec("core"),) * len(out_names),
                      check_rep=False),
            keep_unused=True)

    def zeros_out(self):
        from concourse import mybir
        return [np.zeros((NCORES * a.shape[0], *a.shape[1:]), a.dtype)
                for a in self.out_avals]

    def run(self, xh_concat):
        import jax
        out = self.sharded(xh_concat, *self.zeros_out())
        jax.block_until_ready(out)
        return np.asarray(out[0])


def _weights_sig(ws):
    h = hashlib.md5()
    for a in ws:
        a = np.ascontiguousarray(np.asarray(a))
        h.update(str(a.shape).encode())
        h.update(str(a.dtype).encode())
        h.update(a.tobytes())
    return h.hexdigest()


def get_exec(W1, b1, tau_n, tau_m_h, W2, b2, tau_m_ro, mask, **opts):
    sig = (_weights_sig([W1, b1, tau_n, tau_m_h, W2, b2, tau_m_ro, mask]),
           tuple(sorted(opts.items())))
    ex = _CACHE.get(sig)
    if ex is None:
        C = _prep_consts(W1, b1, tau_n, tau_m_h, W2, b2, tau_m_ro, mask)
        nc = _build_program(C, **opts)
        ex = _Exec(nc)
        _CACHE[sig] = ex
    return ex


def marshal_x(x):
    """(B, T, IN) f32 -> concatenated (NCORES*NSB, NK, BC*TB) uint8 wire tensor."""
    x = np.asarray(x, np.float32)
    xp = np.zeros((B_FULL, TPAD, NK), np.uint8)
    xp[:, :T, :INP] = np.rint(x * np.float32(255.0)).astype(np.uint8)
    xp[:, :, INP] = 255     # "ones"-row for bias (weights carry the /255)
    # (B, TPAD, NK) -> (NCORES, NSB, NK, BC*TB)
    xpc = xp.reshape(NCORES, BC, NSB, TB, NK)
    xh = np.ascontiguousarray(xpc.transpose(0, 2, 4, 1, 3)).reshape(
        NCORES * NSB, NK, BC * TB)
    return xh


def kernel(x, W1, b1, tau_n, tau_m_h, W2, b2, tau_m_ro, mask):
    ex = get_exec(W1, b1, tau_n, tau_m_h, W2, b2, tau_m_ro, mask)
    xh = marshal_x(x)
    out = ex.run(xh)                        # (NCORES*BC, OUT)
    return out.reshape(B_FULL, OUT)


# revision 18
# speedup vs baseline: 1.3148x; 1.1469x over previous
"""DH-SNN network kernel for Trainium2 (8 NeuronCores, batch-parallel).

Wire-optimized: the per-execute cost over the axon tunnel is dominated by
input bytes shipped per call, so:
  - all weights/decay constants are baked into the NEFF via inline_tensor
    (host-precomputed: sigmoid taus, (W1*mask) branch-major reorder,
    2*(1-alpha)*(1-beta_j) prescale, readout time-weights) — shipped once
    at model load, not per execute;
  - only x crosses the wire per call, as uint8 (round(x*255)) in the
    matmul-ready layout [NSB, 701, BC*TB] (700 input rows + a 255-row for
    the bias, T padded to 256); on device it is widened to fp16 (ACT copy)
    and the 1/255 dequant is applied once at the linear branch-sum exit
    (not folded into the fp16 weights, where it would hit denormal flush).

Device math (per core, batch shard Bc=16), unchanged from baseline:
  DI'[f,(b,t)] = xT @ Wm''  (PE fp16 matmuls, features on partitions)
  d-scan over time per (f,b): d = beta_j*d + DI'  (DVE tensor_tensor_scan,
      reset pattern kills state at per-b series boundaries; cross-superblock
      carry injected into the tau=0 column)
  som''[h,(b,t)] = sum_j d  (Pool adds; som'' = 2*(1-alpha)*som)
  v-loop (w = 2*v): w_t = alpha*w_{t-1} - Y_t;  Y_{t+1} = (w_t >= 1) - som''_{t+1}
  readout: out[b,o] = sum_t c_{t,o} * (s_t @ W2.T)[b,o] + b2[o]*sum_t c_{t,o}
      with c_{t,o} = (1 - gam_o^(250-t))/250; linear in s so P = W2T@(Y+som'')
      accumulates on PE, then weighted time-reduce on DVE.
"""

import os
import hashlib
import numpy as np

B_FULL, T, INP, H, NB, OUT = 128, 250, 700, 512, 4, 20
NCORES = 8
BC = B_FULL // NCORES          # 16 batch per core
TB = 32                        # superblock length
NSB = 8                        # superblocks (Tpad = 256)
TPAD = NSB * TB
F = H * NB                     # 2048 features, branch-major
NM = F // 128                  # 16 feature tiles
KC = 6                         # K chunks (700 inputs + 1 ones-row = 701 rows)
KSZ = [128] * 5 + [61]
NK = INP + 1                   # 701
REPS = int(os.environ.get("SNN_REPS", "1"))
ABL = set(os.environ.get("SNN_ABLATE", "").split(","))

_CACHE = {}


def _sigmoid(z):
    return 1.0 / (1.0 + np.exp(-z))


def _prep_consts(W1, b1, tau_n, tau_m_h, W2, b2, tau_m_ro, mask):
    """Host-side: everything that does not depend on x, in float64."""
    f8 = np.float64
    beta = _sigmoid(np.asarray(tau_n, f8).reshape(NB))
    alpha = float(_sigmoid(float(np.asarray(tau_m_h).reshape(()))))
    gam = _sigmoid(np.asarray(tau_m_ro, f8).reshape(OUT))
    sc = 2.0 * (1.0 - alpha) * (1.0 - beta)            # (NB,)

    # feature reorder g=h*NB+j -> f'=j*H+h
    W1r = np.asarray(W1, f8).reshape(H, NB, INP).transpose(1, 0, 2).reshape(F, INP)
    mkr = np.asarray(mask, f8).reshape(H, NB, INP).transpose(1, 0, 2).reshape(F, INP)
    b1r = np.asarray(b1, f8).reshape(H, NB).T.reshape(F)
    scf = np.repeat(sc, H)                             # (F,)
    # x ships as uint8 (round(x*255)) and is DMA-cast to fp16 holding 0..255.
    # Weights stay unscaled (a /255 fold would push ~1/3 of nonzero weights
    # into fp16 denormals, which the PE flushes); the dendritic scan is linear
    # in DI, so the 1/255 dequant is applied once at the branch-sum exit.
    Wfull = np.zeros((NK, F), f8)
    Wfull[:INP] = (W1r * mkr).T * scf[None, :]
    Wfull[INP] = b1r * scf                             # bias row (255-row in x)
    wt16 = Wfull.astype(np.float16)                    # (701, F)

    alpha128 = np.full((128, 1), alpha, np.float32)
    bscale = [np.full((128, 1), beta[j], np.float32) for j in range(NB)]
    # scan reset patterns, one per j-pair: beta_j on its j slice, 0 at tau=0
    pat = []
    for jp in range(2):
        p = np.zeros((128, 2, BC, TB), np.float32)
        for jj in range(2):
            p[:, jj, :, 1:] = beta[2 * jp + jj]
        pat.append(p)

    # readout time weights: column tau of (Y+som'') holds s_{tau-1}; weight
    # by c_{tau-1}: C[tau] = (1 - gam^(T+1-tau))/T for tau in [1, T], else 0
    tau_i = np.arange(TPAD)
    ctau = (1.0 - gam[:, None] ** (T + 1 - tau_i[None, :])) / T
    ctau[:, 0] = 0.0
    ctau[:, T + 1:] = 0.0
    cbig = np.repeat(ctau[:, None, :], BC, axis=1).astype(np.float32)  # (OUT, BC, TPAD)
    b2term = (np.asarray(b2, f8).reshape(OUT) * ctau.sum(1)).reshape(OUT, 1).astype(np.float32)

    w2t = np.ascontiguousarray(np.asarray(W2, np.float32).T).reshape(4, 128, OUT)
    return dict(wt16=wt16, alpha128=alpha128, bscale=bscale, pat=pat,
                cbig=cbig, b2term=b2term, w2t=w2t)


def _build_program(C, cast="act", ro_r=False, vl_split=False, abl=None):
    import concourse.tile as tile
    from concourse import bacc, mybir
    from contextlib import ExitStack

    DT = mybir.dt
    ALU = mybir.AluOpType
    AF = mybir.ActivationFunctionType
    f32 = DT.float32
    f16 = DT.float16

    abl = ABL if abl is None else set(abl)
    nc = bacc.Bacc("TRN2", target_bir_lowering=False, debug=False)

    xh_d = nc.dram_tensor("xh", [NSB, NK, BC * TB], DT.uint8, kind="ExternalInput")
    out_d = nc.dram_tensor("out", [BC, OUT], f32, kind="ExternalOutput")

    wt_d = nc.inline_tensor(C["wt16"], name="wtc")
    a128_d = nc.inline_tensor(C["alpha128"], name="a128c")
    bsc_d = [nc.inline_tensor(C["bscale"][j], name=f"bscc{j}") for j in range(NB)]
    pat_d = [nc.inline_tensor(C["pat"][jp], name=f"patc{jp}") for jp in range(2)]
    cbig_d = nc.inline_tensor(C["cbig"], name="cbigc")
    b2t_d = nc.inline_tensor(C["b2term"], name="b2tc")
    w2t_d = nc.inline_tensor(C["w2t"], name="w2tc")

    with tile.TileContext(nc) as tc, ExitStack() as ctx:
        const = ctx.enter_context(tc.tile_pool(name="const", bufs=1))
        xt_pool = ctx.enter_context(tc.tile_pool(name="xt", bufs=2))
        d_pool = ctx.enter_context(tc.tile_pool(name="dp", bufs=8))
        tmp_pool = ctx.enter_context(tc.tile_pool(name="tp", bufs=3))
        ps_di = ctx.enter_context(tc.tile_pool(name="psdi", bufs=3, space="PSUM"))
        ps_ro = ctx.enter_context(tc.tile_pool(name="psro", bufs=2, space="PSUM"))

        # ------------- constants to SBUF -------------
        wt = []
        for k in range(KC):
            w_ = const.tile([128, F], f16, tag=f"wt{k}", name=f"wt{k}")
            nc.sync.dma_start(w_[0:KSZ[k], :], wt_d.ap()[k * 128:k * 128 + KSZ[k], :])
            wt.append(w_)
        w2t = []
        for c in range(4):
            w2c = const.tile([128, OUT], f32, tag=f"w2t{c}", name=f"w2t{c}")
            nc.sync.dma_start(w2c[:], w2t_d.ap()[c])
            w2t.append(w2c)
        alpha128 = const.tile([128, 1], f32, tag="a128", name="a128")
        nc.sync.dma_start(alpha128[:], a128_d.ap())
        beta128 = []
        for j in range(NB):
            b_ = const.tile([128, 1], f32, tag=f"b128_{j}", name=f"b128_{j}")
            nc.sync.dma_start(b_[:], bsc_d[j].ap())
            beta128.append(b_)
        pat = []
        for jp in range(2):
            p_ = const.tile([128, 2, BC, TB], f32, tag=f"pat{jp}", name=f"pat{jp}")
            nc.sync.dma_start(p_[:], pat_d[jp].ap())
            pat.append(p_)
        cbig = const.tile([OUT, BC, TPAD], f32, tag="cbig", name="cbig")
        nc.sync.dma_start(cbig[:], cbig_d.ap())
        b2term = const.tile([OUT, 1], f32, tag="b2term", name="b2term")
        nc.sync.dma_start(b2term[:], b2t_d.ap())

        # ------------- state -------------
        carry = const.tile([128, NM, BC], f32, tag="carry", name="carry")
        nc.gpsimd.memset(carry[:], 0.0)
        wv = const.tile([128, 4, BC], f32, tag="wv", name="wv")
        nc.gpsimd.memset(wv[:], 0.0)
        acc = const.tile([OUT, BC], f32, tag="acc", name="acc")
        nc.gpsimd.memset(acc[:], 0.0)
        som = [const.tile([128, 4, BC, TB], f32, tag=f"som{i}", name=f"som{i}") for i in range(2)]
        xn = [const.tile([128, 4, BC, TB], f32, tag=f"xn{i}", name=f"xn{i}") for i in range(2)]

        def compute_sb(s):
            """matmuls + scans + branch sum for superblock s."""
            xts = []
            for k in range(KC):
                xt = xt_pool.tile([128, BC * TB], f16, tag=f"xt{k}", name=f"xt{k}")
                if cast == "swdge":
                    # gpsimd (software DGE) casts uint8 -> fp16 in-flight
                    nc.gpsimd.dma_start(xt[0:KSZ[k], :],
                                        xh_d.ap()[s, k * 128:k * 128 + KSZ[k], :])
                else:
                    xu = xt_pool.tile([128, BC * TB], DT.uint8,
                                      tag=f"xu{k}", name=f"xu{k}")
                    nc.sync.dma_start(xu[0:KSZ[k], :],
                                      xh_d.ap()[s, k * 128:k * 128 + KSZ[k], :])
                    if cast == "act":
                        nc.scalar.activation(xt[0:KSZ[k], :], xu[0:KSZ[k], :], AF.Copy)
                    else:
                        nc.vector.tensor_copy(xt[0:KSZ[k], :], xu[0:KSZ[k], :])
                xts.append(xt)
            for c in range(4):
                dt_ = []
                for jp in range(2):
                    pd = ps_di.tile([128, 2, BC * TB], f32, tag="di", name="di")
                    for jj in (() if "mm" in abl else range(2)):
                        m = (2 * jp + jj) * 4 + c
                        for k in range(KC):
                            nc.tensor.matmul(
                                pd[:, jj],
                                lhsT=wt[k][0:KSZ[k], m * 128:(m + 1) * 128],
                                rhs=xts[k][0:KSZ[k], :],
                                start=(k == 0), stop=(k == KC - 1),
                            )
                    pd4 = pd[:].rearrange("p j (b t) -> p j b t", b=BC)
                    # inject beta-prescaled cross-superblock carries (tau=0)
                    m0 = 2 * jp * 4 + c
                    nc.vector.tensor_tensor(
                        pd4[:, :, :, 0], pd4[:, :, :, 0],
                        carry[:, m0:m0 + 5:4, :], ALU.add)
                    d_ = d_pool.tile([128, 2, BC, TB], f32, tag="d", name="d")
                    nc.vector.tensor_tensor_scan(
                        d_[:].rearrange("p j b t -> p (j b t)"),
                        pat[jp][:].rearrange("p j b t -> p (j b t)"),
                        pd[:].rearrange("p j bt -> p (j bt)"),
                        initial=0.0, op0=ALU.mult, op1=ALU.add)
                    for jj in range(2):
                        m = (2 * jp + jj) * 4 + c
                        nc.scalar.activation(carry[:, m, :], d_[:, jj, :, TB - 1],
                                             AF.Copy, scale=beta128[2 * jp + jj][:])
                    dt_.append(d_)
                t01 = tmp_pool.tile([128, BC, TB], f32, tag="t01", name="t01")
                t23 = tmp_pool.tile([128, BC, TB], f32, tag="t23", name="t23")
                nc.gpsimd.tensor_tensor(t01[:], dt_[0][:, 0], dt_[0][:, 1], ALU.add)
                nc.gpsimd.tensor_tensor(t23[:], dt_[1][:, 0], dt_[1][:, 1], ALU.add)
                nc.gpsimd.tensor_tensor(t01[:], t01[:], t23[:], ALU.add)
                # dequant: d carries the 255x from the uint8 x wire format
                nc.gpsimd.tensor_scalar(som[s % 2][:, c], t01[:], 1.0 / 255.0,
                                        None, ALU.mult)

        if vl_split:
            veng = [nc.vector, nc.gpsimd]
            vsl = [slice(0, 2), slice(2, 4)]
        else:
            veng = [nc.vector]
            vsl = [slice(0, 4)]

        def vloop(s):
            if "vloop" in abl:
                return
            if s == 0:
                for eng, cs in zip(veng, vsl):
                    eng.tensor_scalar(xn[0][:, cs, :, 0], som[0][:, cs, :, 0],
                                      -1.0, None, ALU.mult)
            for tl in range(TB):
                t = s * TB + tl
                if t > T - 1:
                    break
                for eng, cs in zip(veng, vsl):
                    eng.scalar_tensor_tensor(
                        wv[:, cs, :], wv[:, cs, :], alpha128[:],
                        xn[s % 2][:, cs, :, tl],
                        op0=ALU.mult, op1=ALU.subtract)
                tn = t + 1
                if tn <= T:
                    s2, tl2 = divmod(tn, TB)
                    for eng, cs in zip(veng, vsl):
                        eng.scalar_tensor_tensor(
                            xn[s2 % 2][:, cs, :, tl2], wv[:, cs, :], 1.0,
                            som[s2 % 2][:, cs, :, tl2],
                            op0=ALU.is_ge, op1=ALU.subtract)

        def readout(s):
            if "ro" in abl:
                return
            P = ps_ro.tile([OUT, BC * TB], f32, tag="P", name="P")
            first = True
            for c in range(4):
                for src in (xn[s % 2], som[s % 2]):
                    s2d = src[:].rearrange("p c b t -> p (c b t)")
                    lhsT = w2t[c][:]
                    rhs = s2d[:, c * BC * TB:(c + 1) * BC * TB]
                    if ro_r:
                        lhsT = lhsT.bitcast(DT.float32r)
                        rhs = rhs.bitcast(DT.float32r)
                    nc.tensor.matmul(
                        P[:], lhsT=lhsT, rhs=rhs,
                        start=first, stop=(c == 3 and src is som[s % 2]))
                    first = False
            p3 = P[:].rearrange("p (b t) -> p b t", b=BC)
            nc.vector.tensor_tensor(
                p3, p3, cbig[:, :, s * TB:(s + 1) * TB], ALU.mult)
            res = tmp_pool.tile([OUT, BC], f32, tag="res", name="res")
            nc.vector.tensor_reduce(res[:], p3, axis=mybir.AxisListType.X,
                                    op=ALU.add)
            nc.vector.tensor_tensor(acc[:], acc[:], res[:], ALU.add)

        for _rep in range(REPS):
            compute_sb(0)
            compute_sb(1)
            for s in range(NSB):
                vloop(s)
                readout(s)
                if s + 2 < NSB:
                    compute_sb(s + 2)

        final = const.tile([OUT, BC], f32, tag="final", name="final")
        nc.vector.tensor_scalar(final[:], acc[:], b2term[:], None, ALU.add)
        nc.sync.dma_start(out_d.ap().rearrange("b o -> o b"), final[:])

    nc.compile()
    return nc


class _Exec:
    """Holds the compiled program and a reusable jitted SPMD dispatcher."""

    def __init__(self, nc):
        import jax
        from jax.sharding import Mesh, PartitionSpec
        from jax.experimental.shard_map import shard_map
        from concourse import bass2jax, mybir

        self.nc = nc
        bass2jax.install_neuronx_cc_hook()
        in_names, out_names, out_avals = [], [], []
        for alloc in nc.m.functions[0].allocations:
            if not isinstance(alloc, mybir.MemoryLocationSet):
                continue
            name = alloc.memorylocations[0].name
            pname = nc.partition_id_tensor.name if nc.partition_id_tensor else None
            if alloc.kind == "ExternalInput":
                if name != pname:
                    in_names.append(name)
            elif alloc.kind == "ExternalOutput":
                out_names.append(name)
                shape = tuple(alloc.tensor_shape)
                dtype = mybir.dt.np(alloc.dtype)
                out_avals.append(jax.core.ShapedArray(shape, dtype))
        self.in_names = list(in_names)
        self.out_names = list(out_names)
        self.out_avals = list(out_avals)
        all_names = in_names + out_names
        if nc.partition_id_tensor is not None:
            all_names = all_names + [nc.partition_id_tensor.name]
        n_io = len(in_names) + len(out_names)
        out_avals_t = tuple(out_avals)
        out_names_t = tuple(out_names)
        all_names_t = tuple(all_names)

        def _body(*args):
            operands = list(args)
            if nc.partition_id_tensor is not None:
                operands.append(bass2jax.partition_id_tensor())
            outs = bass2jax._bass_exec_p.bind(
                *operands, out_avals=out_avals_t, in_names=all_names_t,
                out_names=out_names_t, lowering_input_output_aliases=(),
                sim_require_finite=True, sim_require_nnan=True, nc=nc)
            return tuple(outs)

        devices = jax.devices()[:NCORES]
        self.mesh = Mesh(np.asarray(devices), ("core",))
        self.sharded = jax.jit(
            shard_map(_body, mesh=self.mesh,
                      in_specs=(PartitionSpec("core"),) * n_io,
                      out_specs=(PartitionSpec("core"),) * len(out_names),
                      check_rep=False),
            keep_unused=True)

    def zeros_out(self):
        return [np.zeros((NCORES * a.shape[0], *a.shape[1:]), a.dtype)
                for a in self.out_avals]

    def run(self, xh_concat):
        import jax
        out = self.sharded(xh_concat, *self.zeros_out())
        jax.block_until_ready(out)
        return np.asarray(out[0])


def _weights_sig(ws):
    h = hashlib.md5()
    for a in ws:
        a = np.ascontiguousarray(np.asarray(a))
        h.update(str(a.shape).encode())
        h.update(str(a.dtype).encode())
        h.update(a.tobytes())
    return h.hexdigest()


def get_exec(W1, b1, tau_n, tau_m_h, W2, b2, tau_m_ro, mask, **opts):
    sig = (_weights_sig([W1, b1, tau_n, tau_m_h, W2, b2, tau_m_ro, mask]),
           tuple(sorted(opts.items())))
    ex = _CACHE.get(sig)
    if ex is None:
        C = _prep_consts(W1, b1, tau_n, tau_m_h, W2, b2, tau_m_ro, mask)
        nc = _build_program(C, **opts)
        ex = _Exec(nc)
        _CACHE[sig] = ex
    return ex


def marshal_x(x):
    """(B, T, IN) f32 -> concatenated (NCORES*NSB, NK, BC*TB) uint8 wire tensor."""
    x = np.asarray(x, np.float32)
    xp = np.zeros((B_FULL, TPAD, NK), np.uint8)
    xp[:, :T, :INP] = np.rint(x * np.float32(255.0)).astype(np.uint8)
    xp[:, :, INP] = 255     # "ones"-row for bias (weights carry the /255)
    # (B, TPAD, NK) -> (NCORES, NSB, NK, BC*TB)
    xpc = xp.reshape(NCORES, BC, NSB, TB, NK)
    xh = np.ascontiguousarray(xpc.transpose(0, 2, 4, 1, 3)).reshape(
        NCORES * NSB, NK, BC * TB)
    return xh


def kernel(x, W1, b1, tau_n, tau_m_h, W2, b2, tau_m_ro, mask):
    ex = get_exec(W1, b1, tau_n, tau_m_h, W2, b2, tau_m_ro, mask)
    xh = marshal_x(x)
    out = ex.run(xh)                        # (NCORES*BC, OUT)
    return out.reshape(B_FULL, OUT)
